# revision 13
# baseline (speedup 1.0000x reference)
"""Trainium2 Bass kernel for nn_MetricSelfAttention.

Math (per batch b, head h):
    proj   = X_b @ P_h                         [W, K]
    dots   = (proj @ M_h) @ proj.T             [W, W]   (never formed!)
    nudged = dots @ proj = proj @ (M_h @ (proj.T @ proj))
    out    = concat_h(nudged) @ Wm.T + bias

Two associativity rewrites make this cheap and PE-friendly:
  1. The W x W attention matrix collapses into the K x K Gram matrix
     G = proj.T @ proj  (exact in real arithmetic).
  2. The per-head chain folds right-to-left:  out_h = proj_h @ U_h with
     U_h = (G_h @ M_h).T @ Wm_h.T   (G, M symmetric => A^T = G @ M),
     so `nudged` is never materialized either.

Precision: fp16 matmul inputs with fp32 PSUM accumulation (1 cyc/row on
the PE vs 4 for fp32).  To stay inside fp16 range the projection is
pre-scaled by 1/64 on the host (so proj, G, A carry scales s, s^2, s^2)
and the output partial is scaled by a further 1/64 at eviction; the host
multiplies the summed partials by 64^4 = 2^24 and adds the bias.
End-to-end relative error ~6e-4.

Sharding: 8 cores = 2 batches x 4 head-groups (4 heads each).  Each core
computes a full [W, C] partial of the mixer output for its heads; the
host sums the 4 partials per batch (row-parallel linear "unshard").

Device dataflow per core (P=128, head pairs packed into 128 channels):
  phase 1 (per w-chunk of 512 rows):
    projT[pr][:, wc] = sum_ct  pw[ct,pr].T @ xt[wc][ct]     (PE, psum acc)
    then per 128-w-tile:  transpose back to proj tiles and accumulate
    gram[pr] += proj_t.T @ proj_t                           (PE)
  phase 2:
    AT[pr] = gblk[pr] @ mblk[pr]        (= A^T, block-diagonal)
    U[pr]  = AT[pr].T @ wmt[pr]         [128, 1024]
    out[wt] = (sum_pr projT[pr][:, wt].T @ U[pr]) / 64      (PE + DVE/ACT)
"""

import numpy as np
from contextlib import ExitStack

import concourse.bass as bass
import concourse.mybir as mybir
import concourse.tile as tile
from concourse import bacc
from concourse.bass_utils import run_bass_kernel_spmd

B, W, C, H, K = 2, 2048, 1024, 16, 64
N_CORES = 8
P = 128
WT = W // P          # 16 w-tiles
CT = C // P          # 8 c-tiles
WC = W // 512        # 4 w-chunks
PAIRS = 2            # head pairs per core (4 heads/core)
HK = 4 * K           # 256 channels per core
JBLK = 512
F32 = mybir.dt.float32
F16 = mybir.dt.float16

SCALE_P = 1.0 / 64.0      # folded into the projection weights on host
SCALE_OUT = 1.0 / 64.0    # applied at mixer eviction
RECON = 64.0 ** 4         # host-side reconstruction factor (2^24)

_compiled_nc = None


def _build_kernel():
    from concourse.masks import make_identity

    nc = bacc.Bacc("TRN2", target_bir_lowering=False)

    # xt[wc, ci, ct*512 + w] = X[b, 512*wc + w, 128*ct + ci] * 1.0
    xt = nc.dram_tensor("xt", [WC, P, CT * 512], F16, kind="ExternalInput")
    # pwl[ci, ct*256 + q] = P'[128*ct + ci, q],  q = pair*128 + local chan
    pw = nc.dram_tensor("pw", [P, CT * HK], F16, kind="ExternalInput")
    # host packs pair-major along the free dim: [ci, pr*P + q], [ci, pr*C + j]
    mblk = nc.dram_tensor("mblk", [P, PAIRS * P], F16, kind="ExternalInput")
    wmt = nc.dram_tensor("wmt", [P, PAIRS * C], F16, kind="ExternalInput")
    out = nc.dram_tensor("out", [W, C], F16, kind="ExternalOutput")

    with tile.TileContext(nc) as tc, ExitStack() as ctx:
        const = ctx.enter_context(tc.tile_pool(name="const", bufs=1))

        ident = const.tile([P, P], F16, name="ident", tag="ident")
        make_identity(nc, ident)

        pw_sb = const.tile([P, CT * HK], F16, name="pw_sb", tag="pw_sb")
        nc.sync.dma_start(pw_sb[:], pw[:])

        mblk_sb = const.tile([P, PAIRS * P], F16, name="mblk_sb", tag="mblk_sb")
        nc.scalar.dma_start(mblk_sb[:], mblk[:])
        wmt_sb = const.tile([P, PAIRS * C], F16, name="wmt_sb", tag="wmt_sb")
        nc.scalar.dma_start(wmt_sb[:], wmt[:])

        projT_sb = [
            const.tile([P, W], F16, name=f"projT{pr}", tag=f"projT{pr}")
            for pr in range(PAIRS)
        ]
        gblk_sb = [
            const.tile([P, P], F16, name=f"gblk{pr}", tag=f"gblk{pr}")
            for pr in range(PAIRS)
        ]
        at_sb = [
            const.tile([P, P], F16, name=f"at{pr}", tag=f"at{pr}")
            for pr in range(PAIRS)
        ]
        u_sb = [
            const.tile([P, C], F16, name=f"u{pr}", tag=f"u{pr}")
            for pr in range(PAIRS)
        ]

        outp = ctx.enter_context(tc.tile_pool(name="outp", bufs=3))
        xtp = ctx.enter_context(tc.tile_pool(name="xtp", bufs=3))
        prjp = ctx.enter_context(tc.tile_pool(name="prjp", bufs=6))

        # ---- phase 1: projT + gram, streaming over w-chunks of 512 ----
        with (
            tc.tile_pool(name="pt_ps", bufs=3, space="PSUM") as pt_ps,
            tc.tile_pool(name="gram_ps", bufs=1, space="PSUM") as gram_ps,
            tc.tile_pool(name="tr_ps", bufs=3, space="PSUM") as tr_ps,
        ):
            gps = [
                gram_ps.tile([P, P], F32, name=f"gps{pr}", tag=f"gps{pr}")
                for pr in range(PAIRS)
            ]
            # pre-zero gblk so only the 64x64 block copies remain on the
            # critical gram -> AT path
            for pr in range(PAIRS):
                mslice = mblk_sb[:, pr * P:(pr + 1) * P]
                if pr == 0:
                    nc.vector.tensor_scalar_mul(gblk_sb[pr][:], mslice, 0.0)
                else:
                    nc.scalar.mul(gblk_sb[pr][:], mslice, 0.0)
            for wc in range(WC):
                if wc == 0:
                    # split the first chunk per c-tile so the PE starts
                    # accumulating as soon as the first 128 KB lands
                    xt_parts = []
                    for ct in range(CT):
                        xp = xtp.tile([P, 512], F16, name=f"xt0_{ct}",
                                      tag=f"xt0_{ct}", bufs=1)
                        eng = nc.sync if ct % 2 == 0 else nc.gpsimd
                        eng.dma_start(xp[:], xt[0][:, ct * 512:(ct + 1) * 512])
                        xt_parts.append(xp[:])
                else:
                    xt_t = xtp.tile([P, CT * 512], F16, name="xt_t")
                    nc.sync.dma_start(xt_t[:], xt[wc])
                    xt_parts = [
                        xt_t[:, ct * 512:(ct + 1) * 512] for ct in range(CT)
                    ]
                for pr in range(PAIRS):
                    pt = pt_ps.tile([P, 512], F32, name="pt")
                    for ct in range(CT):
                        nc.tensor.matmul(
                            pt[:],
                            lhsT=pw_sb[:, ct * HK + pr * P:ct * HK + (pr + 1) * P],
                            rhs=xt_parts[ct],
                            start=(ct == 0),
                            stop=(ct == CT - 1),
                        )
                    pslice = projT_sb[pr][:, wc * 512:(wc + 1) * 512]
                    if pr == 0:
                        nc.vector.tensor_copy(pslice, pt[:])
                    else:
                        nc.scalar.copy(pslice, pt[:])
                    # transposes for the 4 w-tiles, batched into one psum
                    # tile and a single eviction copy
                    tps = tr_ps.tile([P, 512], F16, name="tps")
                    for wl in range(4):
                        wt = wc * 4 + wl
                        nc.tensor.transpose(
                            tps[:, wl * P:(wl + 1) * P],
                            projT_sb[pr][:, wt * P:(wt + 1) * P],
                            ident[:],
                        )
                    prj_t = prjp.tile([P, 512], F16, name="prj_t")
                    if (wc + pr) % 2 == 0:
                        nc.vector.tensor_copy(prj_t[:], tps[:])
                    else:
                        nc.scalar.copy(prj_t[:], tps[:])
                    for wl in range(4):
                        wt = wc * 4 + wl
                        nc.tensor.matmul(
                            gps[pr][:],
                            lhsT=prj_t[:, wl * P:(wl + 1) * P],
                            rhs=prj_t[:, wl * P:(wl + 1) * P],
                            start=(wt == 0),
                            stop=(wt == WT - 1),
                        )
            # gram -> block-diag G: only the diagonal 64x64 blocks
            for pr in range(PAIRS):
                eng = nc.vector.tensor_copy if pr == 0 else nc.scalar.copy
                eng(gblk_sb[pr][0:K, 0:K], gps[pr][0:K, 0:K])
                eng(gblk_sb[pr][K:2 * K, K:2 * K], gps[pr][K:2 * K, K:2 * K])

        # ---- phase 2: AT = G@M, U = A.T.T... = AT.T @ wmt, mixer ----
        with (
            tc.tile_pool(name="at_ps", bufs=2, space="PSUM") as at_ps,
            tc.tile_pool(name="u_ps", bufs=2, space="PSUM") as u_ps,
            tc.tile_pool(name="mix_ps", bufs=4, space="PSUM") as mix_ps,
        ):
            for pr in range(PAIRS):
                mslice = mblk_sb[:, pr * P:(pr + 1) * P]
                aps = at_ps.tile([P, P], F32, name="aps")
                # AT = (M @ G)^T = G @ M  (both symmetric, block-diagonal)
                nc.tensor.matmul(
                    aps[:], lhsT=gblk_sb[pr][:], rhs=mslice,
                    start=True, stop=True,
                )
                if pr == 0:
                    nc.scalar.copy(at_sb[pr][:], aps[:])
                else:
                    nc.vector.tensor_copy(at_sb[pr][:], aps[:])
            for pr in range(PAIRS):
                for j in range(C // JBLK):
                    ups = u_ps.tile([P, JBLK], F32, name="ups")
                    nc.tensor.matmul(
                        ups[:],
                        lhsT=at_sb[pr][:],
                        rhs=wmt_sb[:, pr * C + j * JBLK:pr * C + (j + 1) * JBLK],
                        start=True, stop=True,
                    )
                    if (pr + j) % 2 == 0:
                        nc.vector.tensor_copy(
                            u_sb[pr][:, j * JBLK:(j + 1) * JBLK], ups[:]
                        )
                    else:
                        nc.scalar.copy(
                            u_sb[pr][:, j * JBLK:(j + 1) * JBLK], ups[:]
                        )
            for wc in range(WC):
                ob = outp.tile([P, 4 * C], F16, name="ob")
                for wl in range(4):
                    wt = wc * 4 + wl
                    for j in range(C // JBLK):
                        mps = mix_ps.tile([P, JBLK], F32, name="mps")
                        for pr in range(PAIRS):
                            nc.tensor.matmul(
                                mps[:],
                                lhsT=projT_sb[pr][:, wt * P:(wt + 1) * P],
                                rhs=u_sb[pr][:, j * JBLK:(j + 1) * JBLK],
                                start=(pr == 0),
                                stop=(pr == PAIRS - 1),
                            )
                        oslice = ob[:, wl * C + j * JBLK:wl * C + (j + 1) * JBLK]
                        if (wl + j) % 2 == 0:
                            nc.vector.tensor_scalar_mul(oslice, mps[:], SCALE_OUT)
                        else:
                            nc.scalar.mul(oslice, mps[:], SCALE_OUT)
                # one big DMA per 512-row block; issued from gpsimd to keep
                # the sync sequencer free for input descriptor generation
                for half in range(2):
                    dst = out[wc * 512 + half * 256:
                              wc * 512 + (half + 1) * 256, :].rearrange(
                        "(wl p) j -> p wl j", p=P
                    )
                    shalf = ob[:, half * 2 * C:(half + 1) * 2 * C].rearrange(
                        "p (wl j) -> p wl j", wl=2
                    )
                    nc.gpsimd.dma_start(dst, shalf)

    nc.compile()
    return nc


def _get_nc():
    global _compiled_nc
    if _compiled_nc is None:
        _compiled_nc = _build_kernel()
    return _compiled_nc


def _build_metric(halves, diagonals_nk):
    iu, ju = np.triu_indices(K, k=1)
    M = np.zeros((H, K, K), np.float32)
    M[:, iu, ju] = halves
    M = M + M.transpose(0, 2, 1)
    M[:, np.arange(K), np.arange(K)] = diagonals_nk
    return M


def make_in_maps(in_sequence_bwc, projection_1nck, halves, diagonals_nk,
                 mixer_w, mixer_b):
    X = np.asarray(in_sequence_bwc, np.float32)
    Pj = np.asarray(projection_1nck, np.float32)[0]
    M = _build_metric(np.asarray(halves, np.float32),
                      np.asarray(diagonals_nk, np.float32)).astype(np.float16)
    Wm = np.asarray(mixer_w, np.float32)
    Pjs = (Pj * SCALE_P).astype(np.float16)

    xt_b = [
        np.ascontiguousarray(
            X[b].reshape(WC, 512, CT, P).transpose(0, 3, 2, 1).reshape(WC, P, CT * 512)
        ).astype(np.float16)
        for b in range(B)
    ]

    in_maps = []
    for core in range(N_CORES):
        b, g = core // 4, core % 4
        heads = [4 * g + i for i in range(4)]
        # [C, 256] head-major channels -> [ci, ct*256 + q]
        pw_full = Pjs[heads].transpose(1, 0, 2).reshape(C, HK)
        pw_np = np.ascontiguousarray(
            pw_full.reshape(CT, P, HK).transpose(1, 0, 2).reshape(P, CT * HK)
        )
        mblk_np = np.zeros((PAIRS, P, P), np.float16)
        wmt_np = np.empty((PAIRS, P, C), np.float16)
        for pr in range(PAIRS):
            h0, h1 = heads[2 * pr], heads[2 * pr + 1]
            mblk_np[pr, 0:K, 0:K] = M[h0]
            mblk_np[pr, K:2 * K, K:2 * K] = M[h1]
            ch0 = 256 * g + 128 * pr
            wmt_np[pr] = Wm[:, ch0:ch0 + 128].T.astype(np.float16)
        in_maps.append({
            "xt": xt_b[b],
            "pw": pw_np,
            "mblk": np.ascontiguousarray(
                mblk_np.transpose(1, 0, 2).reshape(P, PAIRS * P)
            ),
            "wmt": np.ascontiguousarray(
                wmt_np.transpose(1, 0, 2).reshape(P, PAIRS * C)
            ),
        })
    return in_maps


def kernel(in_sequence_bwc, projection_1nck, halves, diagonals_nk,
           mixer_w, mixer_b, _trace=False, _tmpdir=None):
    nc = _get_nc()
    in_maps = make_in_maps(in_sequence_bwc, projection_1nck, halves,
                           diagonals_nk, mixer_w, mixer_b)
    kwargs = {}
    if _trace:
        kwargs = dict(trace=True, tmpdir=_tmpdir)
    res = run_bass_kernel_spmd(nc, in_maps, core_ids=list(range(N_CORES)),
                               **kwargs)
    acc = np.zeros((B, W, C), np.float32)
    for core in range(N_CORES):
        acc[core // 4] += res.results[core]["out"].astype(np.float32)
    out = (acc * RECON + np.asarray(mixer_b, np.float32)).astype(np.float32)
    if _trace:
        return out, res
    return out


# revision 19
# speedup vs baseline: 1.0537x; 1.0537x over previous
"""Trainium2 Bass kernel for nn_MetricSelfAttention.

Math (per batch b, head h):
    proj   = X_b @ P_h                         [W, K]
    dots   = (proj @ M_h) @ proj.T             [W, W]   (never formed!)
    nudged = dots @ proj = proj @ (M_h @ (proj.T @ proj))
    out    = concat_h(nudged) @ Wm.T + bias

Two associativity rewrites make this cheap and PE-friendly:
  1. The W x W attention matrix collapses into the K x K Gram matrix
     G = proj.T @ proj  (exact in real arithmetic).
  2. The per-head chain folds right-to-left:  out_h = proj_h @ U_h with
     U_h = (G_h @ M_h).T @ Wm_h.T   (G, M symmetric => A^T = G @ M),
     so `nudged` is never materialized either.

Precision: fp16 matmul inputs with fp32 PSUM accumulation (1 cyc/row on
the PE vs 4 for fp32).  To stay inside fp16 range the projection is
pre-scaled by 1/64 on the host (so proj, G, A carry scales s, s^2, s^2)
and the output partial is scaled by a further 1/64 at eviction; the host
multiplies the summed partials by 64^4 = 2^24 and adds the bias.
End-to-end relative error ~6e-4.

Sharding: 8 cores = 2 batches x 4 head-groups (4 heads each).  Each core
computes a full [W, C] partial of the mixer output for its heads; the
host sums the 4 partials per batch (row-parallel linear "unshard").

Device dataflow per core (P=128, head pairs packed into 128 channels):
  phase 1 (per w-chunk of 512 rows):
    projT[pr][:, wc] = sum_ct  pw[ct,pr].T @ xt[wc][ct]     (PE, psum acc)
    then per 128-w-tile:  transpose back to proj tiles and accumulate
    gram[pr] += proj_t.T @ proj_t                           (PE)
  phase 2:
    AT[pr] = gblk[pr] @ mblk[pr]        (= A^T, block-diagonal)
    U[pr]  = AT[pr].T @ wmt[pr]         [128, 1024]
    out[wt] = (sum_pr projT[pr][:, wt].T @ U[pr]) / 64      (PE + DVE/ACT)
"""

import numpy as np
from contextlib import ExitStack

import concourse.bass as bass
import concourse.mybir as mybir
import concourse.tile as tile
from concourse import bacc
from concourse.bass_utils import run_bass_kernel_spmd

B, W, C, H, K = 2, 2048, 1024, 16, 64
N_CORES = 8
P = 128
WT = W // P          # 16 w-tiles
CT = C // P          # 8 c-tiles
WC = W // 512        # 4 w-chunks
PAIRS = 2            # head pairs per core (4 heads/core)
HK = 4 * K           # 256 channels per core
JBLK = 512
F32 = mybir.dt.float32
F16 = mybir.dt.float16

SCALE_P = 1.0 / 64.0      # folded into the projection weights on host
SCALE_OUT = 1.0 / 64.0    # applied at mixer eviction
RECON = 64.0 ** 4         # host-side reconstruction factor (2^24)

_compiled_nc = None


class _FastTileContext(tile.TileContext):
    """TileContext whose epilogue skips the full semaphore-file reset.

    The stock ``_drain_and_barrier`` clears every allocated semaphore
    (~250 individual ops split across engines, ~6-9us on the critical
    path).  The reset only matters for re-executing the NEFF with stale
    semaphore state; the runtime zeroes the semaphore file on each
    ``nrt_execute``, so repeated kernel() calls stay correct (verified
    empirically by repeated runs).
    """

    def _drain_and_barrier(self, tick_clock, wait_clock):
        from concourse.vector_clock import ScopedClock

        drain_inst = self.nc.sync.drain()
        wait_clock.add_sem_waits(
            drain_inst.ins, ScopedClock({None: tick_clock.global_clock})
        )
        self.nc.all_engine_barrier()
        popped = self.nc._tile_sem_poison_stack.pop()
        assert popped is self._sem_poison
        self.nc.all_engine_barrier()


def _build_kernel():
    from concourse.masks import make_identity

    nc = bacc.Bacc("TRN2", target_bir_lowering=False)

    # xt[wc, ci, ct*512 + w] = X[b, 512*wc + w, 128*ct + ci] * 1.0
    xt = nc.dram_tensor("xt", [WC, P, CT * 512], F16, kind="ExternalInput")
    # pwl[ci, ct*256 + q] = P'[128*ct + ci, q],  q = pair*128 + local chan
    pw = nc.dram_tensor("pw", [P, CT * HK], F16, kind="ExternalInput")
    # host packs pair-major along the free dim: [ci, pr*P + q], [ci, pr*C + j]
    mblk = nc.dram_tensor("mblk", [P, PAIRS * P], F16, kind="ExternalInput")
    wmt = nc.dram_tensor("wmt", [P, PAIRS * C], F16, kind="ExternalInput")
    out = nc.dram_tensor("out", [W, C], F16, kind="ExternalOutput")

    with _FastTileContext(nc) as tc, ExitStack() as ctx:
        const = ctx.enter_context(tc.tile_pool(name="const", bufs=1))

        pw_sb = const.tile([P, CT * HK], F16, name="pw_sb", tag="pw_sb")
        nc.sync.dma_start(pw_sb[:], pw[:])

        projT_sb = [
            const.tile([P, W], F16, name=f"projT{pr}", tag=f"projT{pr}")
            for pr in range(PAIRS)
        ]
        gblk_sb = [
            const.tile([P, P], F16, name=f"gblk{pr}", tag=f"gblk{pr}")
            for pr in range(PAIRS)
        ]
        at_sb = [
            const.tile([P, P], F16, name=f"at{pr}", tag=f"at{pr}")
            for pr in range(PAIRS)
        ]
        u_sb = [
            const.tile([P, C], F16, name=f"u{pr}", tag=f"u{pr}")
            for pr in range(PAIRS)
        ]

        outp = ctx.enter_context(tc.tile_pool(name="outp", bufs=3))
        xtp = ctx.enter_context(tc.tile_pool(name="xtp", bufs=3))
        prjp = ctx.enter_context(tc.tile_pool(name="prjp", bufs=6))

        # issue the first-chunk X DMAs before anything else runs on the
        # sync/gpsimd sequencers (make_identity would otherwise delay the
        # gpsimd-issued halves)
        xt0_parts = []
        for ct in range(CT):
            xp = xtp.tile([P, 512], F16, name=f"xt0_{ct}",
                          tag=f"xt0_{ct}", bufs=1)
            eng = nc.sync if ct % 2 == 0 else nc.gpsimd
            eng.dma_start(xp[:], xt[0][:, ct * 512:(ct + 1) * 512])
            xt0_parts.append(xp[:])

        ident = const.tile([P, P], F16, name="ident", tag="ident")
        make_identity(nc, ident)

        mblk_sb = const.tile([P, PAIRS * P], F16, name="mblk_sb", tag="mblk_sb")
        nc.scalar.dma_start(mblk_sb[:], mblk[:])
        wmt_sb = const.tile([P, PAIRS * C], F16, name="wmt_sb", tag="wmt_sb")
        nc.scalar.dma_start(wmt_sb[:], wmt[:])

        # ---- phase 1: projT + gram, streaming over w-chunks of 512 ----
        with (
            tc.tile_pool(name="pt_ps", bufs=3, space="PSUM") as pt_ps,
            tc.tile_pool(name="gram_ps", bufs=1, space="PSUM") as gram_ps,
            tc.tile_pool(name="tr_ps", bufs=3, space="PSUM") as tr_ps,
        ):
            gps = [
                gram_ps.tile([P, P], F32, name=f"gps{pr}", tag=f"gps{pr}")
                for pr in range(PAIRS)
            ]
            # pre-zero gblk so only the 64x64 block copies remain on the
            # critical gram -> AT path
            for pr in range(PAIRS):
                mslice = mblk_sb[:, pr * P:(pr + 1) * P]
                if pr == 0:
                    nc.vector.tensor_scalar_mul(gblk_sb[pr][:], mslice, 0.0)
                else:
                    nc.scalar.mul(gblk_sb[pr][:], mslice, 0.0)
            for wc in range(WC):
                if wc == 0:
                    # first chunk was issued per c-tile up front
                    xt_parts = xt0_parts
                else:
                    xt_t = xtp.tile([P, CT * 512], F16, name="xt_t")
                    nc.sync.dma_start(xt_t[:], xt[wc])
                    xt_parts = [
                        xt_t[:, ct * 512:(ct + 1) * 512] for ct in range(CT)
                    ]
                for pr in range(PAIRS):
                    pt = pt_ps.tile([P, 512], F32, name="pt")
                    for ct in range(CT):
                        nc.tensor.matmul(
                            pt[:],
                            lhsT=pw_sb[:, ct * HK + pr * P:ct * HK + (pr + 1) * P],
                            rhs=xt_parts[ct],
                            start=(ct == 0),
                            stop=(ct == CT - 1),
                        )
                    pslice = projT_sb[pr][:, wc * 512:(wc + 1) * 512]
                    if pr == 0:
                        nc.vector.tensor_copy(pslice, pt[:])
                    else:
                        nc.scalar.copy(pslice, pt[:])
                    # transposes for the 4 w-tiles, batched into one psum
                    # tile and a single eviction copy
                    tps = tr_ps.tile([P, 512], F16, name="tps")
                    for wl in range(4):
                        wt = wc * 4 + wl
                        nc.tensor.transpose(
                            tps[:, wl * P:(wl + 1) * P],
                            projT_sb[pr][:, wt * P:(wt + 1) * P],
                            ident[:],
                        )
                    prj_t = prjp.tile([P, 512], F16, name="prj_t")
                    if (wc + pr) % 2 == 0:
                        nc.vector.tensor_copy(prj_t[:], tps[:])
                    else:
                        nc.scalar.copy(prj_t[:], tps[:])
                    for wl in range(4):
                        wt = wc * 4 + wl
                        nc.tensor.matmul(
                            gps[pr][:],
                            lhsT=prj_t[:, wl * P:(wl + 1) * P],
                            rhs=prj_t[:, wl * P:(wl + 1) * P],
                            start=(wt == 0),
                            stop=(wt == WT - 1),
                        )
            # gram -> block-diag G: only the diagonal 64x64 blocks
            for pr in range(PAIRS):
                eng = nc.vector.tensor_copy if pr == 0 else nc.scalar.copy
                eng(gblk_sb[pr][0:K, 0:K], gps[pr][0:K, 0:K])
                eng(gblk_sb[pr][K:2 * K, K:2 * K], gps[pr][K:2 * K, K:2 * K])

        # ---- phase 2: AT = G@M, U = A.T.T... = AT.T @ wmt, mixer ----
        with (
            tc.tile_pool(name="at_ps", bufs=1, space="PSUM") as at_ps,
            tc.tile_pool(name="u_ps", bufs=2, space="PSUM") as u_ps,
            tc.tile_pool(name="mix_ps", bufs=5, space="PSUM") as mix_ps,
        ):
            for pr in range(PAIRS):
                mslice = mblk_sb[:, pr * P:(pr + 1) * P]
                aps = at_ps.tile([P, P], F32, name="aps")
                # AT = (M @ G)^T = G @ M  (both symmetric, block-diagonal)
                nc.tensor.matmul(
                    aps[:], lhsT=gblk_sb[pr][:], rhs=mslice,
                    start=True, stop=True,
                )
                if pr == 0:
                    nc.scalar.copy(at_sb[pr][:], aps[:])
                else:
                    nc.vector.tensor_copy(at_sb[pr][:], aps[:])
            for pr in range(PAIRS):
                for j in range(C // JBLK):
                    ups = u_ps.tile([P, JBLK], F32, name="ups")
                    nc.tensor.matmul(
                        ups[:],
                        lhsT=at_sb[pr][:],
                        rhs=wmt_sb[:, pr * C + j * JBLK:pr * C + (j + 1) * JBLK],
                        start=True, stop=True,
                    )
                    if (pr + j) % 2 == 0:
                        nc.vector.tensor_copy(
                            u_sb[pr][:, j * JBLK:(j + 1) * JBLK], ups[:]
                        )
                    else:
                        nc.scalar.copy(
                            u_sb[pr][:, j * JBLK:(j + 1) * JBLK], ups[:]
                        )
            for wc in range(WC):
                ob = outp.tile([P, 4 * C], F16, name="ob")
                for wl in range(4):
                    wt = wc * 4 + wl
                    for j in range(C // JBLK):
                        mps = mix_ps.tile([P, JBLK], F32, name="mps")
                        for pr in range(PAIRS):
                            nc.tensor.matmul(
                                mps[:],
                                lhsT=projT_sb[pr][:, wt * P:(wt + 1) * P],
                                rhs=u_sb[pr][:, j * JBLK:(j + 1) * JBLK],
                                start=(pr == 0),
                                stop=(pr == PAIRS - 1),
                            )
                        oslice = ob[:, wl * C + j * JBLK:wl * C + (j + 1) * JBLK]
                        if (wl + j) % 2 == 0:
                            nc.vector.tensor_scalar_mul(oslice, mps[:], SCALE_OUT)
                        else:
                            nc.scalar.mul(oslice, mps[:], SCALE_OUT)
                # one big DMA per 512-row block; issued from gpsimd to keep
                # the sync sequencer free for input descriptor generation
                for half in range(2):
                    dst = out[wc * 512 + half * 256:
                              wc * 512 + (half + 1) * 256, :].rearrange(
                        "(wl p) j -> p wl j", p=P
                    )
                    shalf = ob[:, half * 2 * C:(half + 1) * 2 * C].rearrange(
                        "p (wl j) -> p wl j", wl=2
                    )
                    nc.gpsimd.dma_start(dst, shalf)

    nc.compile()
    return nc


def _get_nc():
    global _compiled_nc
    if _compiled_nc is None:
        _compiled_nc = _build_kernel()
    return _compiled_nc


def _build_metric(halves, diagonals_nk):
    iu, ju = np.triu_indices(K, k=1)
    M = np.zeros((H, K, K), np.float32)
    M[:, iu, ju] = halves
    M = M + M.transpose(0, 2, 1)
    M[:, np.arange(K), np.arange(K)] = diagonals_nk
    return M


def make_in_maps(in_sequence_bwc, projection_1nck, halves, diagonals_nk,
                 mixer_w, mixer_b):
    X = np.asarray(in_sequence_bwc, np.float32)
    Pj = np.asarray(projection_1nck, np.float32)[0]
    M = _build_metric(np.asarray(halves, np.float32),
                      np.asarray(diagonals_nk, np.float32)).astype(np.float16)
    Wm = np.asarray(mixer_w, np.float32)
    Pjs = (Pj * SCALE_P).astype(np.float16)

    xt_b = [
        np.ascontiguousarray(
            X[b].reshape(WC, 512, CT, P).transpose(0, 3, 2, 1).reshape(WC, P, CT * 512)
        ).astype(np.float16)
        for b in range(B)
    ]

    in_maps = []
    for core in range(N_CORES):
        b, g = core // 4, core % 4
        heads = [4 * g + i for i in range(4)]
        # [C, 256] head-major channels -> [ci, ct*256 + q]
        pw_full = Pjs[heads].transpose(1, 0, 2).reshape(C, HK)
        pw_np = np.ascontiguousarray(
            pw_full.reshape(CT, P, HK).transpose(1, 0, 2).reshape(P, CT * HK)
        )
        mblk_np = np.zeros((PAIRS, P, P), np.float16)
        wmt_np = np.empty((PAIRS, P, C), np.float16)
        for pr in range(PAIRS):
            h0, h1 = heads[2 * pr], heads[2 * pr + 1]
            mblk_np[pr, 0:K, 0:K] = M[h0]
            mblk_np[pr, K:2 * K, K:2 * K] = M[h1]
            ch0 = 256 * g + 128 * pr
            wmt_np[pr] = Wm[:, ch0:ch0 + 128].T.astype(np.float16)
        in_maps.append({
            "xt": xt_b[b],
            "pw": pw_np,
            "mblk": np.ascontiguousarray(
                mblk_np.transpose(1, 0, 2).reshape(P, PAIRS * P)
            ),
            "wmt": np.ascontiguousarray(
                wmt_np.transpose(1, 0, 2).reshape(P, PAIRS * C)
            ),
        })
    return in_maps


def kernel(in_sequence_bwc, projection_1nck, halves, diagonals_nk,
           mixer_w, mixer_b, _trace=False, _tmpdir=None):
    nc = _get_nc()
    in_maps = make_in_maps(in_sequence_bwc, projection_1nck, halves,
                           diagonals_nk, mixer_w, mixer_b)
    kwargs = {}
    if _trace:
        kwargs = dict(trace=True, tmpdir=_tmpdir)
    res = run_bass_kernel_spmd(nc, in_maps, core_ids=list(range(N_CORES)),
                               **kwargs)
    acc = np.zeros((B, W, C), np.float32)
    for core in range(N_CORES):
        acc[core // 4] += res.results[core]["out"].astype(np.float32)
    out = (acc * RECON + np.asarray(mixer_b, np.float32)).astype(np.float32)
    if _trace:
        return out, res
    return out
